# revision 7
# baseline (speedup 1.0000x reference)
"""Trainium2 Bass kernel for nn_CometBertECTagging (B=64, L=512, HB=768, HC=1024, NL=7).

Reference computation (per batch row i):
  pos  = cumsum(valid[i]) - 1
  valid_output[i, pos[j]] = bert[i, j]  if valid[i, j] == 1; other slots zero
  logits[i] = concat([valid_output[i], comet[i]], -1) @ W + b

Device algorithm (data-parallel over batch, 8 rows per core):
  - logits = compact(bert @ Wb) + comet @ Wc + b: compaction applied to the
    per-token bert logits [L, 7] instead of the bert activations [L, 768].
  - compaction as matmul: S[j, slot] = (valid[j]*cumsum[j]-1 == slot), built on
    DVE via is_equal against an iota row (bf16: 0/1 exact); compacted_logits^T
    accumulates into the same PSUM tile as the comet logits.
  - cumsum(valid) via matmul with an upper-triangular ones matrix (exact).
  - ALL activations loaded upfront: one SWDGE cast-DMA (f32->bf16 during the
    HBM read) per (row, tensor) = 16 big DMAs (1.6/2.1 MB each) queued on the
    Pool ring at t=0; SBUF holds all 8 rows (112 KB/partition). No per-tile
    HWDGE loads, no DVE/ACT cast traffic.
  - per-row: h-tiles transposed by normal-mode identity matmuls (bf16 weights
    -> FWL fast weight load), PSUM evicted bf16 alternating DVE/ACT, then bf16
    classifier matmuls with W-slice stationary contract over h into [7, L]
    PSUM. Bert logits go [7,L] -> transpose-mode matmul -> [L,7] bf16 ->
    scatter matmul against bf16 S tiles into the same PSUM.
  - output stored transposed [7, L] per row (2KB/partition DMA writes); host
    untransposes during unshard. Tiny-chunk DMAs (<512B/partition) are
    avoided everywhere: they scramble partitions on this DMA path.
"""

import numpy as np

import concourse.bacc as bacc
import concourse.mybir as mybir
from concourse.tile import TileContext
from concourse.bass_utils import run_bass_kernel_spmd

F32 = mybir.dt.float32
F32R = mybir.dt.float32r
BF16 = mybir.dt.bfloat16
I32 = mybir.dt.int32

B, L, HB, HC, NL = 64, 512, 768, 1024, 7
NCORES = 8
RPC = B // NCORES  # batch rows per core
JT = L // 128      # j tiles per row
HTB = HB // 128    # bert h tiles
HTC = HC // 128    # comet h tiles

_PROGRAM = None


def build_program(loop_iters=None):
    nc = bacc.Bacc(target_bir_lowering=False)

    bert = nc.dram_tensor("bert", [RPC, L, HB], F32, kind="ExternalInput")
    comet = nc.dram_tensor("comet", [RPC, L, HC], F32, kind="ExternalInput")
    validn = nc.dram_tensor("validn", [128, L], I32, kind="ExternalInput")
    wtile = nc.dram_tensor("wtile", [128, 128], F32, kind="ExternalInput")
    biasr = nc.dram_tensor("biasr", [NL, L], F32, kind="ExternalInput")
    out = nc.dram_tensor("out", [RPC, NL, L], F32, kind="ExternalOutput")

    iota_np = np.broadcast_to(np.arange(L, dtype=np.float32), (128, L)).copy()
    triu_np = np.triu(np.ones((L, L), dtype=np.float32))  # T[j, j'] = 1 iff j <= j'
    ident_np = np.eye(128, dtype=np.float32)
    iota_c = nc.inline_tensor(iota_np, name="iota_c")
    triu_c = nc.inline_tensor(triu_np, name="triu_c")
    ident_c = nc.inline_tensor(ident_np, name="ident_c")

    with TileContext(nc) as tc:
        with (
            tc.tile_pool(name="const", bufs=1) as cpool,
            tc.tile_pool(name="nat", bufs=RPC) as npool,
            tc.tile_pool(name="txp", bufs=6) as tpool,
            tc.tile_pool(name="sel", bufs=2) as sel_pool,
            tc.tile_pool(name="small", bufs=2) as smpool,
            tc.tile_pool(name="ps_t", bufs=3, space="PSUM") as pt_pool,
            tc.tile_pool(name="ps_l", bufs=2, space="PSUM") as pl_pool,
            tc.tile_pool(name="ps_s", bufs=1, space="PSUM") as ps_pool,
        ):

            def body():
                # ---- upfront activation loads: 16 big SWDGE cast DMAs ----
                natb_tiles, natc_tiles = [], []
                for r in range(RPC):
                    nat_b = npool.tile([128, JT * HB], BF16, tag="nat_b", name="nat_b")
                    nc.gpsimd.dma_start(
                        out=nat_b[:].rearrange("p (t h) -> p t h", h=HB),
                        in_=bert[r].rearrange("(t p) h -> p t h", p=128),
                    )
                    nat_c = npool.tile([128, JT * HC], BF16, tag="nat_c", name="nat_c")
                    nc.gpsimd.dma_start(
                        out=nat_c[:].rearrange("p (t h) -> p t h", h=HC),
                        in_=comet[r].rearrange("(t p) h -> p t h", p=128),
                    )
                    natb_tiles.append(nat_b)
                    natc_tiles.append(nat_c)

                # ---- constants / setup ----
                iota_sb = cpool.tile([128, L], F32, name="iota_sb")
                nc.sync.dma_start(out=iota_sb[:], in_=iota_c[:])
                tri_sb = cpool.tile([128, JT * L], F32R, name="tri_sb")
                for a in range(JT):
                    nc.sync.dma_start(
                        out=tri_sb[:, a * L : (a + 1) * L],
                        in_=triu_c[a * 128 : (a + 1) * 128, :].bitcast(F32R),
                    )
                ident32_sb = cpool.tile([128, 128], F32, name="ident32_sb")
                nc.sync.dma_start(out=ident32_sb[:], in_=ident_c[:])
                identb_sb = cpool.tile([128, 128], BF16, name="identb_sb")
                nc.vector.tensor_copy(out=identb_sb[:], in_=ident32_sb[:])
                w32_sb = cpool.tile([128, 128], F32, name="w32_sb")
                nc.sync.dma_start(out=w32_sb[:], in_=wtile[:])
                w_sb = cpool.tile([128, 128], BF16, name="w_sb")
                nc.vector.tensor_copy(out=w_sb[:], in_=w32_sb[:])
                bias_sb = cpool.tile([NL, L], F32, name="bias_sb")
                nc.sync.dma_start(out=bias_sb[:], in_=biasr[:])
                vrawn_sb = cpool.tile([128, L], I32, name="vrawn_sb")
                nc.sync.dma_start(out=vrawn_sb[:], in_=validn[:])
                vfn_sb = cpool.tile([128, L], F32, name="vfn_sb")
                nc.vector.tensor_copy(out=vfn_sb[:], in_=vrawn_sb[:])
                vf0_sb = cpool.tile([128, JT * RPC], F32, name="vf0_sb")
                for a in range(JT):
                    vt_ps = pt_pool.tile([128, 128], F32, tag="ps_t", name="vt_ps")
                    nc.tensor.matmul(
                        out=vt_ps[:],
                        lhsT=vfn_sb[:, a * 128 : (a + 1) * 128],
                        rhs=ident32_sb[:],
                        is_transpose=True,
                        start=True,
                        stop=True,
                    )
                    nc.vector.tensor_copy(
                        out=vf0_sb[:, a * RPC : (a + 1) * RPC], in_=vt_ps[:, :RPC]
                    )
                vf_sb = cpool.tile([128, JT * RPC], F32R, name="vf_sb")
                nc.vector.tensor_copy(out=vf_sb[:], in_=vf0_sb[:])

                # cumsum over L per row: C[j', r] = sum_{j<=j'} valid[j, r]
                cs_ps = ps_pool.tile([128, JT * RPC], F32, tag="ps_s", name="cs_ps")
                n_mm = sum(kt + 1 for kt in range(JT))
                i_mm = 0
                for mt in range(JT):
                    for kt in range(mt + 1):
                        nc.tensor.matmul(
                            out=cs_ps[:, mt * RPC : (mt + 1) * RPC],
                            lhsT=tri_sb[:, kt * L + mt * 128 : kt * L + (mt + 1) * 128],
                            rhs=vf_sb[:, kt * RPC : (kt + 1) * RPC],
                            start=(i_mm == 0),
                            stop=(i_mm == n_mm - 1),
                        )
                        i_mm += 1
                mval_sb = cpool.tile([128, JT * RPC], F32, name="mval_sb")
                nc.vector.tensor_mul(out=mval_sb[:], in0=vf0_sb[:], in1=cs_ps[:])
                nc.vector.tensor_scalar_add(mval_sb[:], mval_sb[:], -1.0)

                # ---- per-row pipeline ----
                for r in range(RPC):
                    nat_b = natb_tiles[r]
                    nat_c = natc_tiles[r]

                    # selection matrix tiles S[j_local, slot] per j-tile (bf16)
                    s_tiles = []
                    for jt in range(JT):
                        s_t = sel_pool.tile([128, L], BF16, tag=f"s{jt}", name="s_t")
                        nc.vector.tensor_scalar(
                            out=s_t[:],
                            in0=iota_sb[:],
                            scalar1=mval_sb[:, jt * RPC + r : jt * RPC + r + 1],
                            scalar2=None,
                            op0=mybir.AluOpType.is_equal,
                        )
                        s_tiles.append(s_t)

                    # bert: transpose h-tiles (identity matmul), classifier
                    evict = 0
                    psum_bl = pl_pool.tile([NL, L], F32, tag="ps_bl", name="psum_bl")
                    for ht in range(HTB):
                        pt = pt_pool.tile([128, L], F32, tag="ps_t", name="pt")
                        for jt in range(JT):
                            nc.tensor.matmul(
                                out=pt[:, jt * 128 : (jt + 1) * 128],
                                lhsT=nat_b[:, jt * HB + ht * 128 : jt * HB + (ht + 1) * 128],
                                rhs=identb_sb[:],
                                start=(jt == 0),
                                stop=(jt == JT - 1),
                            )
                        tb = tpool.tile([128, L], BF16, tag="txp", name="tb")
                        if evict % 2 == 0:
                            nc.vector.tensor_copy(out=tb[:], in_=pt[:])
                        else:
                            nc.scalar.copy(out=tb[:], in_=pt[:])
                        evict += 1
                        nc.tensor.matmul(
                            out=psum_bl[:],
                            lhsT=w_sb[:, ht * NL : (ht + 1) * NL],
                            rhs=tb[:],
                            start=(ht == 0),
                            stop=(ht == HTB - 1),
                        )

                    # comet: transpose h-tiles, classifier into psum_fin
                    psum_fin = pl_pool.tile([NL, L], F32, tag="ps_fin", name="psum_fin")
                    for ht in range(HTC):
                        pt = pt_pool.tile([128, L], F32, tag="ps_t", name="pt")
                        for jt in range(JT):
                            nc.tensor.matmul(
                                out=pt[:, jt * 128 : (jt + 1) * 128],
                                lhsT=nat_c[:, jt * HC + ht * 128 : jt * HC + (ht + 1) * 128],
                                rhs=identb_sb[:],
                                start=(jt == 0),
                                stop=(jt == JT - 1),
                            )
                        tcm = tpool.tile([128, L], BF16, tag="txp", name="tcm")
                        if evict % 2 == 0:
                            nc.vector.tensor_copy(out=tcm[:], in_=pt[:])
                        else:
                            nc.scalar.copy(out=tcm[:], in_=pt[:])
                        evict += 1
                        nc.tensor.matmul(
                            out=psum_fin[:],
                            lhsT=w_sb[:, (HTB + ht) * NL : (HTB + ht + 1) * NL],
                            rhs=tcm[:],
                            start=(ht == 0),
                            stop=False,
                        )

                    # bert logits: evict, transpose to [j, 7], scatter via S
                    bl_sb = smpool.tile([NL, L], F32, tag="bl", name="bl_sb")
                    nc.scalar.copy(out=bl_sb[:], in_=psum_bl[:])
                    blt_ps = ps_pool.tile([128, JT * NL], F32, tag="ps_s", name="blt_ps")
                    for jt in range(JT):
                        nc.tensor.matmul(
                            out=blt_ps[:, jt * NL : (jt + 1) * NL],
                            lhsT=bl_sb[:, jt * 128 : (jt + 1) * 128],
                            rhs=ident32_sb[:NL, :NL],
                            is_transpose=True,
                            start=(jt == 0),
                            stop=(jt == JT - 1),
                        )
                    NLP = 8  # bf16 copy of blt, NL padded to 8 for 4B-aligned slices
                    blt_sb = smpool.tile([128, JT * NLP], BF16, tag="blt", name="blt_sb")
                    nc.vector.tensor_copy(
                        out=blt_sb[:].rearrange("p (t n) -> p t n", n=NLP)[:, :, :NL],
                        in_=blt_ps[:].rearrange("p (t n) -> p t n", n=NL),
                    )
                    for jt in range(JT):
                        nc.tensor.matmul(
                            out=psum_fin[:],
                            lhsT=blt_sb[:, jt * NLP : jt * NLP + NL],
                            rhs=s_tiles[jt][:],
                            start=False,
                            stop=(jt == JT - 1),
                        )

                    # bias add; store transposed [7, L] (host untransposes)
                    fin_sb = smpool.tile([NL, L], F32, tag="fin", name="fin_sb")
                    nc.scalar.add(fin_sb[:], psum_fin[:], bias_sb[:, 0:1])
                    nc.sync.dma_start(out=out[r], in_=fin_sb[:])

            if loop_iters is None:
                body()
            else:
                with tc.For_i(0, loop_iters, 1):
                    body()

    nc.compile()
    return nc


def build_baseline_program():
    nc = bacc.Bacc(target_bir_lowering=False)
    nc.dram_tensor("bert", [RPC, L, HB], F32, kind="ExternalInput")
    nc.dram_tensor("comet", [RPC, L, HC], F32, kind="ExternalInput")
    nc.dram_tensor("validn", [128, L], I32, kind="ExternalInput")
    nc.dram_tensor("wtile", [128, 128], F32, kind="ExternalInput")
    biasr = nc.dram_tensor("biasr", [NL, L], F32, kind="ExternalInput")
    out = nc.dram_tensor("out", [RPC, NL, L], F32, kind="ExternalOutput")
    with TileContext(nc) as tc:
        with tc.tile_pool(name="sb", bufs=2) as pool:
            t = pool.tile([NL, L], F32)
            nc.sync.dma_start(out=t[:], in_=biasr[:])
            for r in range(RPC):
                nc.sync.dma_start(out=out[r], in_=t[:])
    nc.compile()
    return nc


def get_program():
    global _PROGRAM
    if _PROGRAM is None:
        _PROGRAM = build_program()
    return _PROGRAM


def make_in_maps(bert, comet, valid, w, b):
    bert = np.ascontiguousarray(np.asarray(bert, dtype=np.float32))
    comet = np.ascontiguousarray(np.asarray(comet, dtype=np.float32))
    valid = np.asarray(valid, dtype=np.int32)
    w = np.ascontiguousarray(np.asarray(w, dtype=np.float32))
    b = np.asarray(b, dtype=np.float32).reshape(NL, 1)
    b_rep = np.ascontiguousarray(np.broadcast_to(b, (NL, L)))
    w_tiled = np.zeros((128, 128), dtype=np.float32)
    w_tiled[:, : (HB + HC) // 128 * NL] = (
        w.reshape((HB + HC) // 128, 128, NL).transpose(1, 0, 2).reshape(128, -1)
    )
    in_maps = []
    for c in range(NCORES):
        rows = slice(c * RPC, (c + 1) * RPC)
        in_maps.append(
            {
                "bert": np.ascontiguousarray(bert[rows]),
                "comet": np.ascontiguousarray(comet[rows]),
                "validn": np.concatenate(
                    [valid[rows], np.zeros((128 - RPC, L), np.int32)], axis=0
                ),
                "wtile": w_tiled,
                "biasr": b_rep,
            }
        )
    return in_maps


def kernel(
    bert_sequence_output, comet_sequence_output, valid_ids, classifier_w, classifier_b
):
    nc = get_program()
    in_maps = make_in_maps(
        bert_sequence_output, comet_sequence_output, valid_ids, classifier_w, classifier_b
    )
    res = run_bass_kernel_spmd(nc, in_maps, list(range(NCORES)))
    return np.concatenate(
        [res.results[c]["out"].transpose(0, 2, 1) for c in range(NCORES)], axis=0
    )


if __name__ == "__main__":
    rng = np.random.default_rng(0)
    ins = {
        "bert_sequence_output": rng.standard_normal((B, L, HB), dtype=np.float32),
        "comet_sequence_output": rng.standard_normal((B, L, HC), dtype=np.float32),
        "valid_ids": rng.integers(0, 2, size=(B, L), dtype=np.int32),
        "classifier_w": (rng.standard_normal((HB + HC, NL)) * 0.02).astype(np.float32),
        "classifier_b": (rng.standard_normal((NL,)) * 0.02).astype(np.float32),
    }
    got = kernel(**ins)
    print("kernel output:", got.shape, got.dtype)


# revision 9
# speedup vs baseline: 1.1761x; 1.1761x over previous
"""Trainium2 Bass kernel for nn_CometBertECTagging (B=64, L=512, HB=768, HC=1024, NL=7).

Reference computation (per batch row i):
  pos  = cumsum(valid[i]) - 1
  valid_output[i, pos[j]] = bert[i, j]  if valid[i, j] == 1; other slots zero
  logits[i] = concat([valid_output[i], comet[i]], -1) @ W + b

Device algorithm (data-parallel over batch, 8 rows per core):
  - logits = compact(bert @ Wb) + comet @ Wc + b: compaction applied to the
    per-token bert logits [L, 7] instead of the bert activations [L, 768].
  - compaction as matmul: S[j, slot] = (valid[j]*cumsum[j]-1 == slot), built on
    DVE via is_equal against an iota row (bf16: 0/1 exact); compacted_logits^T
    accumulates into the same PSUM tile as the comet logits.
  - cumsum(valid) via matmul with an upper-triangular ones matrix (exact).
  - ALL activations loaded upfront: one SWDGE cast-DMA (f32->bf16 during the
    HBM read) per (row, tensor) = 16 big DMAs (1.6/2.1 MB each) queued on the
    Pool ring at t=0; SBUF holds all 8 rows (112 KB/partition). No per-tile
    HWDGE loads, no DVE/ACT cast traffic.
  - per-row: h-tiles transposed by normal-mode identity matmuls (bf16 weights
    -> FWL fast weight load), PSUM evicted bf16 alternating DVE/ACT, then bf16
    classifier matmuls with W-slice stationary contract over h into [7, L]
    PSUM. Bert logits go [7,L] -> transpose-mode matmul -> [L,7] bf16 ->
    scatter matmul against bf16 S tiles into the same PSUM.
  - output stored transposed [7, L] per row (2KB/partition DMA writes); host
    untransposes during unshard. Tiny-chunk DMAs (<512B/partition) are
    avoided everywhere: they scramble partitions on this DMA path.
"""

import numpy as np

import concourse.bacc as bacc
import concourse.mybir as mybir
from concourse.tile import TileContext
from concourse.bass_utils import run_bass_kernel_spmd

F32 = mybir.dt.float32
F32R = mybir.dt.float32r
BF16 = mybir.dt.bfloat16
FP16 = mybir.dt.float16
I32 = mybir.dt.int32

B, L, HB, HC, NL = 64, 512, 768, 1024, 7
NCORES = 8
RPC = B // NCORES  # batch rows per core
JT = L // 128      # j tiles per row
HTB = HB // 128    # bert h tiles
HTC = HC // 128    # comet h tiles

_PROGRAM = None


def build_program(loop_iters=None):
    nc = bacc.Bacc(target_bir_lowering=False)

    bert = nc.dram_tensor("bert", [RPC, L, HB], F32, kind="ExternalInput")
    comet = nc.dram_tensor("comet", [RPC, L, HC], F32, kind="ExternalInput")
    validn = nc.dram_tensor("validn", [128, L], I32, kind="ExternalInput")
    wtile = nc.dram_tensor("wtile", [128, 128], F32, kind="ExternalInput")
    biasr = nc.dram_tensor("biasr", [NL, L], F32, kind="ExternalInput")
    out = nc.dram_tensor("out", [RPC, NL, L], F32, kind="ExternalOutput")

    iota_np = np.broadcast_to(np.arange(L, dtype=np.float32), (128, L)).copy()
    triu_np = np.triu(np.ones((L, L), dtype=np.float16))  # T[j, j'] = 1 iff j <= j'
    ident_np = np.eye(128, dtype=np.float32)
    iota_c = nc.inline_tensor(iota_np, name="iota_c")
    triu_c = nc.inline_tensor(triu_np, name="triu_c")
    ident_c = nc.inline_tensor(ident_np, name="ident_c")

    with TileContext(nc) as tc:
        with (
            tc.tile_pool(name="const", bufs=1) as cpool,
            tc.tile_pool(name="nat", bufs=RPC) as npool,
            tc.tile_pool(name="txp", bufs=6) as tpool,
            tc.tile_pool(name="sel", bufs=2) as sel_pool,
            tc.tile_pool(name="small", bufs=2) as smpool,
            tc.tile_pool(name="ps_t", bufs=3, space="PSUM") as pt_pool,
            tc.tile_pool(name="ps_l", bufs=2, space="PSUM") as pl_pool,
            tc.tile_pool(name="ps_s", bufs=1, space="PSUM") as ps_pool,
        ):

            def body():
                # ---- upfront activation loads: 16 big SWDGE cast DMAs ----
                natb_tiles, natc_tiles = [], []
                for r in range(RPC):
                    nat_b = npool.tile([128, JT * HB], BF16, tag="nat_b", name="nat_b")
                    nc.gpsimd.dma_start(
                        out=nat_b[:].rearrange("p (t h) -> p t h", h=HB),
                        in_=bert[r].rearrange("(t p) h -> p t h", p=128),
                    )
                    nat_c = npool.tile([128, JT * HC], BF16, tag="nat_c", name="nat_c")
                    nc.gpsimd.dma_start(
                        out=nat_c[:].rearrange("p (t h) -> p t h", h=HC),
                        in_=comet[r].rearrange("(t p) h -> p t h", p=128),
                    )
                    natb_tiles.append(nat_b)
                    natc_tiles.append(nat_c)

                # ---- constants / setup ----
                iota_sb = cpool.tile([128, L], F32, name="iota_sb")
                nc.sync.dma_start(out=iota_sb[:], in_=iota_c[:])
                tri_sb = cpool.tile([128, JT * L], FP16, name="tri_sb")
                for a in range(JT):
                    nc.sync.dma_start(
                        out=tri_sb[:, a * L : (a + 1) * L],
                        in_=triu_c[a * 128 : (a + 1) * 128, :],
                    )
                ident32_sb = cpool.tile([128, 128], F32, name="ident32_sb")
                nc.sync.dma_start(out=ident32_sb[:], in_=ident_c[:])
                identb_sb = cpool.tile([128, 128], BF16, name="identb_sb")
                nc.vector.tensor_copy(out=identb_sb[:], in_=ident32_sb[:])
                w32_sb = cpool.tile([128, 128], F32, name="w32_sb")
                nc.sync.dma_start(out=w32_sb[:], in_=wtile[:])
                w_sb = cpool.tile([128, 128], BF16, name="w_sb")
                nc.vector.tensor_copy(out=w_sb[:], in_=w32_sb[:])
                bias_sb = cpool.tile([NL, L], F32, name="bias_sb")
                nc.sync.dma_start(out=bias_sb[:], in_=biasr[:])
                vrawn_sb = cpool.tile([128, L], I32, name="vrawn_sb")
                nc.sync.dma_start(out=vrawn_sb[:], in_=validn[:])
                vfn_sb = cpool.tile([128, L], F32, name="vfn_sb")
                nc.vector.tensor_copy(out=vfn_sb[:], in_=vrawn_sb[:])
                vf0_sb = cpool.tile([128, JT * RPC], F32, name="vf0_sb")
                for a in range(JT):
                    vt_ps = pt_pool.tile([128, 128], F32, tag="ps_t", name="vt_ps")
                    nc.tensor.matmul(
                        out=vt_ps[:],
                        lhsT=vfn_sb[:, a * 128 : (a + 1) * 128],
                        rhs=ident32_sb[:],
                        is_transpose=True,
                        start=True,
                        stop=True,
                    )
                    nc.vector.tensor_copy(
                        out=vf0_sb[:, a * RPC : (a + 1) * RPC], in_=vt_ps[:, :RPC]
                    )
                vf_sb = cpool.tile([128, JT * RPC], FP16, name="vf_sb")
                nc.vector.tensor_copy(out=vf_sb[:], in_=vf0_sb[:])

                # cumsum over L per row: C[j', r] = sum_{j<=j'} valid[j, r]
                cs_ps = ps_pool.tile([128, JT * RPC], F32, tag="ps_s", name="cs_ps")
                n_mm = sum(kt + 1 for kt in range(JT))
                i_mm = 0
                for mt in range(JT):
                    for kt in range(mt + 1):
                        nc.tensor.matmul(
                            out=cs_ps[:, mt * RPC : (mt + 1) * RPC],
                            lhsT=tri_sb[:, kt * L + mt * 128 : kt * L + (mt + 1) * 128],
                            rhs=vf_sb[:, kt * RPC : (kt + 1) * RPC],
                            start=(i_mm == 0),
                            stop=(i_mm == n_mm - 1),
                        )
                        i_mm += 1
                mval_sb = cpool.tile([128, JT * RPC], F32, name="mval_sb")
                nc.vector.tensor_mul(out=mval_sb[:], in0=vf0_sb[:], in1=cs_ps[:])
                nc.vector.tensor_scalar_add(mval_sb[:], mval_sb[:], -1.0)

                # ---- per-row pipeline ----
                for r in range(RPC):
                    nat_b = natb_tiles[r]
                    nat_c = natc_tiles[r]

                    # selection matrix tiles S[j_local, slot] per j-tile (bf16)
                    s_tiles = []
                    for jt in range(JT):
                        s_t = sel_pool.tile([128, L], BF16, tag=f"s{jt}", name="s_t")
                        nc.vector.tensor_scalar(
                            out=s_t[:],
                            in0=iota_sb[:],
                            scalar1=mval_sb[:, jt * RPC + r : jt * RPC + r + 1],
                            scalar2=None,
                            op0=mybir.AluOpType.is_equal,
                        )
                        s_tiles.append(s_t)

                    # bert: transpose h-tiles (identity matmul); then per-j-tile
                    # classifier MMs [j, 8] with the transposed tile as weights
                    NLP = 8
                    evict = 0
                    blt_ps = ps_pool.tile([128, JT * NLP], F32, tag="ps_s", name="blt_ps")
                    for ht in range(HTB):
                        pt = pt_pool.tile([128, L], F32, tag="ps_t", name="pt")
                        for jt in range(JT):
                            nc.tensor.matmul(
                                out=pt[:, jt * 128 : (jt + 1) * 128],
                                lhsT=nat_b[:, jt * HB + ht * 128 : jt * HB + (ht + 1) * 128],
                                rhs=identb_sb[:],
                                start=(jt == 0),
                                stop=(jt == JT - 1),
                            )
                        tb = tpool.tile([128, L], BF16, tag="txp", name="tb")
                        if evict % 2 == 0:
                            nc.vector.tensor_copy(out=tb[:], in_=pt[:])
                        else:
                            nc.scalar.copy(out=tb[:], in_=pt[:])
                        evict += 1
                        for jt in range(JT):
                            nc.tensor.matmul(
                                out=blt_ps[:, jt * NLP : (jt + 1) * NLP],
                                lhsT=tb[:, jt * 128 : (jt + 1) * 128],
                                rhs=w_sb[:, ht * NLP : (ht + 1) * NLP],
                                start=(ht == 0 and jt == 0),
                                stop=(ht == HTB - 1 and jt == JT - 1),
                            )

                    # comet: transpose h-tiles, classifier into psum_fin
                    psum_fin = pl_pool.tile([NL, L], F32, tag="ps_fin", name="psum_fin")
                    for ht in range(HTC):
                        pt = pt_pool.tile([128, L], F32, tag="ps_t", name="pt")
                        for jt in range(JT):
                            nc.tensor.matmul(
                                out=pt[:, jt * 128 : (jt + 1) * 128],
                                lhsT=nat_c[:, jt * HC + ht * 128 : jt * HC + (ht + 1) * 128],
                                rhs=identb_sb[:],
                                start=(jt == 0),
                                stop=(jt == JT - 1),
                            )
                        tcm = tpool.tile([128, L], BF16, tag="txp", name="tcm")
                        if evict % 2 == 0:
                            nc.vector.tensor_copy(out=tcm[:], in_=pt[:])
                        else:
                            nc.scalar.copy(out=tcm[:], in_=pt[:])
                        evict += 1
                        nc.tensor.matmul(
                            out=psum_fin[:],
                            lhsT=w_sb[:, (HTB + ht) * NLP : (HTB + ht) * NLP + NL],
                            rhs=tcm[:],
                            start=(ht == 0),
                            stop=False,
                        )

                    # bert logits: evict [j, 8] tiles bf16, scatter via S
                    blt_sb = smpool.tile([128, JT * NLP], BF16, tag="blt", name="blt_sb")
                    nc.vector.tensor_copy(out=blt_sb[:], in_=blt_ps[:])
                    for jt in range(JT):
                        nc.tensor.matmul(
                            out=psum_fin[:],
                            lhsT=blt_sb[:, jt * NLP : jt * NLP + NL],
                            rhs=s_tiles[jt][:],
                            start=False,
                            stop=(jt == JT - 1),
                        )

                    # bias add; store transposed [7, L] (host untransposes)
                    fin_sb = smpool.tile([NL, L], F32, tag="fin", name="fin_sb")
                    nc.scalar.add(fin_sb[:], psum_fin[:], bias_sb[:, 0:1])
                    nc.sync.dma_start(out=out[r], in_=fin_sb[:])

            if loop_iters is None:
                body()
            else:
                with tc.For_i(0, loop_iters, 1):
                    body()

    nc.compile()
    return nc


def build_baseline_program():
    nc = bacc.Bacc(target_bir_lowering=False)
    nc.dram_tensor("bert", [RPC, L, HB], F32, kind="ExternalInput")
    nc.dram_tensor("comet", [RPC, L, HC], F32, kind="ExternalInput")
    nc.dram_tensor("validn", [128, L], I32, kind="ExternalInput")
    nc.dram_tensor("wtile", [128, 128], F32, kind="ExternalInput")
    biasr = nc.dram_tensor("biasr", [NL, L], F32, kind="ExternalInput")
    out = nc.dram_tensor("out", [RPC, NL, L], F32, kind="ExternalOutput")
    with TileContext(nc) as tc:
        with tc.tile_pool(name="sb", bufs=2) as pool:
            t = pool.tile([NL, L], F32)
            nc.sync.dma_start(out=t[:], in_=biasr[:])
            for r in range(RPC):
                nc.sync.dma_start(out=out[r], in_=t[:])
    nc.compile()
    return nc


def get_program():
    global _PROGRAM
    if _PROGRAM is None:
        _PROGRAM = build_program()
    return _PROGRAM


def make_in_maps(bert, comet, valid, w, b):
    bert = np.ascontiguousarray(np.asarray(bert, dtype=np.float32))
    comet = np.ascontiguousarray(np.asarray(comet, dtype=np.float32))
    valid = np.asarray(valid, dtype=np.int32)
    w = np.ascontiguousarray(np.asarray(w, dtype=np.float32))
    b = np.asarray(b, dtype=np.float32).reshape(NL, 1)
    b_rep = np.ascontiguousarray(np.broadcast_to(b, (NL, L)))
    w_tiled = np.zeros((128, 128), dtype=np.float32)
    wp = np.zeros(((HB + HC) // 128, 128, 8), dtype=np.float32)
    wp[:, :, :NL] = w.reshape((HB + HC) // 128, 128, NL)
    w_tiled[:, : (HB + HC) // 128 * 8] = wp.transpose(1, 0, 2).reshape(128, -1)
    in_maps = []
    for c in range(NCORES):
        rows = slice(c * RPC, (c + 1) * RPC)
        in_maps.append(
            {
                "bert": np.ascontiguousarray(bert[rows]),
                "comet": np.ascontiguousarray(comet[rows]),
                "validn": np.concatenate(
                    [valid[rows], np.zeros((128 - RPC, L), np.int32)], axis=0
                ),
                "wtile": w_tiled,
                "biasr": b_rep,
            }
        )
    return in_maps


def kernel(
    bert_sequence_output, comet_sequence_output, valid_ids, classifier_w, classifier_b
):
    nc = get_program()
    in_maps = make_in_maps(
        bert_sequence_output, comet_sequence_output, valid_ids, classifier_w, classifier_b
    )
    res = run_bass_kernel_spmd(nc, in_maps, list(range(NCORES)))
    return np.concatenate(
        [res.results[c]["out"].transpose(0, 2, 1) for c in range(NCORES)], axis=0
    )


if __name__ == "__main__":
    rng = np.random.default_rng(0)
    ins = {
        "bert_sequence_output": rng.standard_normal((B, L, HB), dtype=np.float32),
        "comet_sequence_output": rng.standard_normal((B, L, HC), dtype=np.float32),
        "valid_ids": rng.integers(0, 2, size=(B, L), dtype=np.int32),
        "classifier_w": (rng.standard_normal((HB + HC, NL)) * 0.02).astype(np.float32),
        "classifier_b": (rng.standard_normal((NL,)) * 0.02).astype(np.float32),
    }
    got = kernel(**ins)
    print("kernel output:", got.shape, got.dtype)


# revision 11
# speedup vs baseline: 1.2027x; 1.0226x over previous
"""Trainium2 Bass kernel for nn_CometBertECTagging (B=64, L=512, HB=768, HC=1024, NL=7).

Reference computation (per batch row i):
  pos  = cumsum(valid[i]) - 1
  valid_output[i, pos[j]] = bert[i, j]  if valid[i, j] == 1; other slots zero
  logits[i] = concat([valid_output[i], comet[i]], -1) @ W + b

Device algorithm (data-parallel over batch, 8 rows per core):
  - logits = compact(bert @ Wb) + comet @ Wc + b: compaction applied to the
    per-token bert logits [L, 7] instead of the bert activations [L, 768].
  - compaction as matmul: S[j, slot] = (valid[j]*cumsum[j]-1 == slot), built on
    DVE via is_equal against an iota row (bf16: 0/1 exact); compacted_logits^T
    accumulates into the same PSUM tile as the comet logits.
  - cumsum(valid) via matmul with an upper-triangular ones matrix (exact).
  - ALL activations loaded upfront: one SWDGE cast-DMA (f32->bf16 during the
    HBM read) per (row, tensor) = 16 big DMAs (1.6/2.1 MB each) queued on the
    Pool ring at t=0; SBUF holds all 8 rows (112 KB/partition). No per-tile
    HWDGE loads, no DVE/ACT cast traffic.
  - per-row: h-tiles transposed by normal-mode identity matmuls (bf16 weights
    -> FWL fast weight load), PSUM evicted bf16 alternating DVE/ACT, then bf16
    classifier matmuls with W-slice stationary contract over h into [7, L]
    PSUM. Bert logits go [7,L] -> transpose-mode matmul -> [L,7] bf16 ->
    scatter matmul against bf16 S tiles into the same PSUM.
  - output stored transposed [7, L] per row (2KB/partition DMA writes); host
    untransposes during unshard. Tiny-chunk DMAs (<512B/partition) are
    avoided everywhere: they scramble partitions on this DMA path.
"""

import numpy as np

import concourse.bacc as bacc
import concourse.mybir as mybir
from concourse.tile import TileContext
from concourse.bass_utils import run_bass_kernel_spmd

F32 = mybir.dt.float32
F32R = mybir.dt.float32r
BF16 = mybir.dt.bfloat16
FP16 = mybir.dt.float16
I32 = mybir.dt.int32

B, L, HB, HC, NL = 64, 512, 768, 1024, 7
NCORES = 8
RPC = B // NCORES  # batch rows per core
JT = L // 128      # j tiles per row
HTB = HB // 128    # bert h tiles
HTC = HC // 128    # comet h tiles

_PROGRAM = None


def build_program(loop_iters=None):
    nc = bacc.Bacc(target_bir_lowering=False)

    bert = nc.dram_tensor("bert", [RPC, L, HB], F32, kind="ExternalInput")
    comet = nc.dram_tensor("comet", [RPC, L, HC], F32, kind="ExternalInput")
    validn = nc.dram_tensor("validn", [128, L], I32, kind="ExternalInput")
    wtile = nc.dram_tensor("wtile", [128, 128], F32, kind="ExternalInput")
    biasr = nc.dram_tensor("biasr", [NL, L], F32, kind="ExternalInput")
    out = nc.dram_tensor("out", [RPC, NL, L], F32, kind="ExternalOutput")

    iota_np = np.broadcast_to(np.arange(L, dtype=np.float16), (128, L)).copy()
    trid_np = np.triu(np.ones((128, 128), dtype=np.float16))  # within-block j<=j'
    ones_np = np.ones((128, 128), dtype=np.float16)           # whole-block j<j'
    ident_np = np.eye(128, dtype=np.float32)
    iota_c = nc.inline_tensor(iota_np, name="iota_c")
    trid_c = nc.inline_tensor(trid_np, name="trid_c")
    ones_c = nc.inline_tensor(ones_np, name="ones_c")
    ident_c = nc.inline_tensor(ident_np, name="ident_c")

    with TileContext(nc) as tc:
        with (
            tc.tile_pool(name="const", bufs=1) as cpool,
            tc.tile_pool(name="nat", bufs=RPC) as npool,
            tc.tile_pool(name="txp", bufs=6) as tpool,
            tc.tile_pool(name="sel", bufs=2) as sel_pool,
            tc.tile_pool(name="small", bufs=2) as smpool,
            tc.tile_pool(name="ps_t", bufs=5, space="PSUM") as pt_pool,
            tc.tile_pool(name="ps_l", bufs=2, space="PSUM") as pl_pool,
            tc.tile_pool(name="ps_s", bufs=1, space="PSUM") as ps_pool,
        ):

            def body():
                # ---- upfront activation loads: 16 big SWDGE cast DMAs ----
                natb_tiles, natc_tiles = [], []
                for r in range(RPC):
                    nat_b = npool.tile([128, JT * HB], BF16, tag="nat_b", name="nat_b")
                    for hh in range(2):
                        hs = slice(hh * (HB // 2), (hh + 1) * (HB // 2))
                        nc.gpsimd.dma_start(
                            out=nat_b[:].rearrange("p (t h) -> p t h", h=HB)[:, :, hs],
                            in_=bert[r].rearrange("(t p) h -> p t h", p=128)[:, :, hs],
                        )
                    nat_c = npool.tile([128, JT * HC], BF16, tag="nat_c", name="nat_c")
                    for hh in range(2):
                        hs = slice(hh * (HC // 2), (hh + 1) * (HC // 2))
                        nc.gpsimd.dma_start(
                            out=nat_c[:].rearrange("p (t h) -> p t h", h=HC)[:, :, hs],
                            in_=comet[r].rearrange("(t p) h -> p t h", p=128)[:, :, hs],
                        )
                    natb_tiles.append(nat_b)
                    natc_tiles.append(nat_c)

                # ---- constants / setup ----
                iota_sb = cpool.tile([128, L], FP16, name="iota_sb")
                nc.sync.dma_start(out=iota_sb[:], in_=iota_c[:])
                trid_sb = cpool.tile([128, 128], FP16, name="trid_sb")
                nc.sync.dma_start(out=trid_sb[:], in_=trid_c[:])
                ones_sb = cpool.tile([128, 128], FP16, name="ones_sb")
                nc.sync.dma_start(out=ones_sb[:], in_=ones_c[:])
                ident32_sb = cpool.tile([128, 128], F32, name="ident32_sb")
                nc.sync.dma_start(out=ident32_sb[:], in_=ident_c[:])
                identb_sb = cpool.tile([128, 128], BF16, name="identb_sb")
                nc.vector.tensor_copy(out=identb_sb[:], in_=ident32_sb[:])
                w32_sb = cpool.tile([128, 128], F32, name="w32_sb")
                nc.sync.dma_start(out=w32_sb[:], in_=wtile[:])
                w_sb = cpool.tile([128, 128], BF16, name="w_sb")
                nc.vector.tensor_copy(out=w_sb[:], in_=w32_sb[:])
                bias_sb = cpool.tile([NL, L], F32, name="bias_sb")
                nc.sync.dma_start(out=bias_sb[:], in_=biasr[:])
                vrawn_sb = cpool.tile([128, L], I32, name="vrawn_sb")
                nc.sync.dma_start(out=vrawn_sb[:], in_=validn[:])
                vfn_sb = cpool.tile([128, L], F32, name="vfn_sb")
                nc.vector.tensor_copy(out=vfn_sb[:], in_=vrawn_sb[:])
                vf0_sb = cpool.tile([128, JT * RPC], F32, name="vf0_sb")
                for a in range(JT):
                    vt_ps = pt_pool.tile([128, 128], F32, tag="ps_t", name="vt_ps")
                    nc.tensor.matmul(
                        out=vt_ps[:],
                        lhsT=vfn_sb[:, a * 128 : (a + 1) * 128],
                        rhs=ident32_sb[:],
                        is_transpose=True,
                        start=True,
                        stop=True,
                    )
                    nc.vector.tensor_copy(
                        out=vf0_sb[:, a * RPC : (a + 1) * RPC], in_=vt_ps[:, :RPC]
                    )
                vf_sb = cpool.tile([128, JT * RPC], FP16, name="vf_sb")
                nc.vector.tensor_copy(out=vf_sb[:], in_=vf0_sb[:])

                # cumsum over L per row: C[j', r] = sum_{j<=j'} valid[j, r]
                cs_ps = ps_pool.tile([128, JT * RPC], F32, tag="ps_s", name="cs_ps")
                n_mm = sum(kt + 1 for kt in range(JT))
                i_mm = 0
                for mt in range(JT):
                    for kt in range(mt + 1):
                        nc.tensor.matmul(
                            out=cs_ps[:, mt * RPC : (mt + 1) * RPC],
                            lhsT=(trid_sb[:] if kt == mt else ones_sb[:]),
                            rhs=vf_sb[:, kt * RPC : (kt + 1) * RPC],
                            start=(i_mm == 0),
                            stop=(i_mm == n_mm - 1),
                        )
                        i_mm += 1
                mval_sb = cpool.tile([128, JT * RPC], F32, name="mval_sb")
                nc.vector.tensor_mul(out=mval_sb[:], in0=vf0_sb[:], in1=cs_ps[:])
                nc.vector.tensor_scalar_add(mval_sb[:], mval_sb[:], -1.0)

                # ---- per-row pipeline ----
                for r in range(RPC):
                    nat_b = natb_tiles[r]
                    nat_c = natc_tiles[r]

                    # selection matrix tiles S[j_local, slot] per j-tile (bf16)
                    s_tiles = []
                    for jt in range(JT):
                        s_t = sel_pool.tile([128, L], BF16, tag=f"s{jt}", name="s_t")
                        nc.vector.tensor_scalar(
                            out=s_t[:],
                            in0=iota_sb[:],
                            scalar1=mval_sb[:, jt * RPC + r : jt * RPC + r + 1],
                            scalar2=None,
                            op0=mybir.AluOpType.is_equal,
                        )
                        s_tiles.append(s_t)

                    # bert: transpose h-tiles (identity matmul); then per-j-tile
                    # classifier MMs [j, 8] with the transposed tile as weights
                    NLP = 8
                    evict = 0
                    blt_ps = ps_pool.tile([128, JT * NLP], F32, tag="ps_s", name="blt_ps")
                    for ht in range(HTB):
                        pt = pt_pool.tile([128, L], F32, tag="ps_t", name="pt")
                        for jt in range(JT):
                            nc.tensor.matmul(
                                out=pt[:, jt * 128 : (jt + 1) * 128],
                                lhsT=nat_b[:, jt * HB + ht * 128 : jt * HB + (ht + 1) * 128],
                                rhs=identb_sb[:],
                                start=(jt == 0),
                                stop=(jt == JT - 1),
                            )
                        tb = tpool.tile([128, L], BF16, tag="txp", name="tb")
                        if evict % 2 == 0:
                            nc.vector.tensor_copy(out=tb[:], in_=pt[:])
                        else:
                            nc.scalar.copy(out=tb[:], in_=pt[:])
                        evict += 1
                        for jt in range(JT):
                            nc.tensor.matmul(
                                out=blt_ps[:, jt * NLP : (jt + 1) * NLP],
                                lhsT=tb[:, jt * 128 : (jt + 1) * 128],
                                rhs=w_sb[:, ht * NLP : (ht + 1) * NLP],
                                start=(ht == 0 and jt == 0),
                                stop=(ht == HTB - 1 and jt == JT - 1),
                            )

                    # comet: transpose h-tiles, classifier into psum_fin
                    psum_fin = pl_pool.tile([NL, L], F32, tag="ps_fin", name="psum_fin")
                    for ht in range(HTC):
                        pt = pt_pool.tile([128, L], F32, tag="ps_t", name="pt")
                        for jt in range(JT):
                            nc.tensor.matmul(
                                out=pt[:, jt * 128 : (jt + 1) * 128],
                                lhsT=nat_c[:, jt * HC + ht * 128 : jt * HC + (ht + 1) * 128],
                                rhs=identb_sb[:],
                                start=(jt == 0),
                                stop=(jt == JT - 1),
                            )
                        tcm = tpool.tile([128, L], BF16, tag="txp", name="tcm")
                        if evict % 2 == 0:
                            nc.vector.tensor_copy(out=tcm[:], in_=pt[:])
                        else:
                            nc.scalar.copy(out=tcm[:], in_=pt[:])
                        evict += 1
                        nc.tensor.matmul(
                            out=psum_fin[:],
                            lhsT=w_sb[:, (HTB + ht) * NLP : (HTB + ht) * NLP + NL],
                            rhs=tcm[:],
                            start=(ht == 0),
                            stop=False,
                        )

                    # bert logits: evict [j, 8] tiles bf16, scatter via S
                    blt_sb = smpool.tile([128, JT * NLP], BF16, tag="blt", name="blt_sb")
                    nc.vector.tensor_copy(out=blt_sb[:], in_=blt_ps[:])
                    for jt in range(JT):
                        nc.tensor.matmul(
                            out=psum_fin[:],
                            lhsT=blt_sb[:, jt * NLP : jt * NLP + NL],
                            rhs=s_tiles[jt][:],
                            start=False,
                            stop=(jt == JT - 1),
                        )

                    # bias add; store transposed [7, L] (host untransposes)
                    fin_sb = smpool.tile([NL, L], F32, tag="fin", name="fin_sb")
                    nc.scalar.add(fin_sb[:], psum_fin[:], bias_sb[:, 0:1])
                    nc.sync.dma_start(out=out[r], in_=fin_sb[:])

            if loop_iters is None:
                body()
            else:
                with tc.For_i(0, loop_iters, 1):
                    body()

    nc.compile()
    return nc


def build_baseline_program():
    nc = bacc.Bacc(target_bir_lowering=False)
    nc.dram_tensor("bert", [RPC, L, HB], F32, kind="ExternalInput")
    nc.dram_tensor("comet", [RPC, L, HC], F32, kind="ExternalInput")
    nc.dram_tensor("validn", [128, L], I32, kind="ExternalInput")
    nc.dram_tensor("wtile", [128, 128], F32, kind="ExternalInput")
    biasr = nc.dram_tensor("biasr", [NL, L], F32, kind="ExternalInput")
    out = nc.dram_tensor("out", [RPC, NL, L], F32, kind="ExternalOutput")
    with TileContext(nc) as tc:
        with tc.tile_pool(name="sb", bufs=2) as pool:
            t = pool.tile([NL, L], F32)
            nc.sync.dma_start(out=t[:], in_=biasr[:])
            for r in range(RPC):
                nc.sync.dma_start(out=out[r], in_=t[:])
    nc.compile()
    return nc


def get_program():
    global _PROGRAM
    if _PROGRAM is None:
        _PROGRAM = build_program()
    return _PROGRAM


def make_in_maps(bert, comet, valid, w, b):
    bert = np.ascontiguousarray(np.asarray(bert, dtype=np.float32))
    comet = np.ascontiguousarray(np.asarray(comet, dtype=np.float32))
    valid = np.asarray(valid, dtype=np.int32)
    w = np.ascontiguousarray(np.asarray(w, dtype=np.float32))
    b = np.asarray(b, dtype=np.float32).reshape(NL, 1)
    b_rep = np.ascontiguousarray(np.broadcast_to(b, (NL, L)))
    w_tiled = np.zeros((128, 128), dtype=np.float32)
    wp = np.zeros(((HB + HC) // 128, 128, 8), dtype=np.float32)
    wp[:, :, :NL] = w.reshape((HB + HC) // 128, 128, NL)
    w_tiled[:, : (HB + HC) // 128 * 8] = wp.transpose(1, 0, 2).reshape(128, -1)
    in_maps = []
    for c in range(NCORES):
        rows = slice(c * RPC, (c + 1) * RPC)
        in_maps.append(
            {
                "bert": np.ascontiguousarray(bert[rows]),
                "comet": np.ascontiguousarray(comet[rows]),
                "validn": np.concatenate(
                    [valid[rows], np.zeros((128 - RPC, L), np.int32)], axis=0
                ),
                "wtile": w_tiled,
                "biasr": b_rep,
            }
        )
    return in_maps


def kernel(
    bert_sequence_output, comet_sequence_output, valid_ids, classifier_w, classifier_b
):
    nc = get_program()
    in_maps = make_in_maps(
        bert_sequence_output, comet_sequence_output, valid_ids, classifier_w, classifier_b
    )
    res = run_bass_kernel_spmd(nc, in_maps, list(range(NCORES)))
    return np.concatenate(
        [res.results[c]["out"].transpose(0, 2, 1) for c in range(NCORES)], axis=0
    )


if __name__ == "__main__":
    rng = np.random.default_rng(0)
    ins = {
        "bert_sequence_output": rng.standard_normal((B, L, HB), dtype=np.float32),
        "comet_sequence_output": rng.standard_normal((B, L, HC), dtype=np.float32),
        "valid_ids": rng.integers(0, 2, size=(B, L), dtype=np.int32),
        "classifier_w": (rng.standard_normal((HB + HC, NL)) * 0.02).astype(np.float32),
        "classifier_b": (rng.standard_normal((NL,)) * 0.02).astype(np.float32),
    }
    got = kernel(**ins)
    print("kernel output:", got.shape, got.dtype)
